# revision 1
# baseline (speedup 1.0000x reference)
"""Trainium2 Bass kernel: Atom2Residue (gnn_message_passing).

Math: out[n,c,o] = sum_i fuse[n,c,i] * w[l(c),o,i]  (+ b[o] at c==0)
where fuse[n,c,:] = concat(CA_atom[n,c,:16], res_emb[n,c,:32]), l(c)=floor(sqrt(c)).

Strategy (8 cores, data parallel over residues, no collectives):
  - Host pre-packs a channel-major bf16 image ft[432, 18750] per core
    (residues on the free axis), so the device needs NO transposes:
      rows   0:128  res channels of coefs 0-3   (G0)
      rows 128:256  res channels of coefs 4-7   (G1)
      rows 256:384  atom channels of coefs 0-7  (G2)
      rows 384:432  res c8 (32) | atom c8 (16)  (G3)
  - Device: per 512-residue tile, 5 weight-stationary matmuls
    (block-diagonal per-coefficient weights, PSUM-accumulated res+atom
    contributions), PSUM -> SBUF bf16 copies, DMA out a transposed
    bf16 output image ot[288, 18750].
  - Host un-transposes the output, casts to f32, adds the l=0 bias.
  - bf16 halves HBM traffic (~27 MB/core); rel-err ~3e-3 << 2e-2 gate.
  - DMAs batched in 4096-column groups (0.4-1 MB per transfer);
    inputs on the SP HWDGE ring, outputs on the ACT ring; triple-buffered.
  - No column padding: exact 18750 residues/core (last tile N=318).

Measured (n_rep-slope wall-clock method, see kernel_timed): ~91 us/exec,
vs 390 us for the f32 PE-transpose baseline and a 75 us bf16 HBM
roofline (27 MB/core at 358 GB/s).
"""

import os
import sys

for _p in ("/opt/trn_rl_repo",):
    if os.path.isdir(_p) and _p not in sys.path:
        sys.path.insert(0, _p)

import numpy as np
from ml_dtypes import bfloat16

from concourse import bacc, bass, mybir  # noqa: F401
from concourse.bass_utils import run_bass_kernel_spmd
from concourse.tile import TileContext

F32 = mybir.dt.float32
BF16 = mybir.dt.bfloat16

NUM_COEF, ATOM_C, NODE_C = 9, 16, 32
L_OF_COEF = np.floor(np.sqrt(np.arange(NUM_COEF))).astype(np.int64)

N_CORES = 8
R_TOTAL = 150_000
RS = R_TOTAL // N_CORES      # 18750 residues per core
TILE = 512                   # residues per matmul tile (PSUM bank = 512 f32)
RC = RS                      # exact columns per core (no padding)
GCOLS = 8 * TILE             # max columns per DMA group (4096)


def _group_widths(rc):
    """Column widths per DMA group. First/last groups are a single tile so
    the unoverlapped pipeline head (first in-DMA) and tail (last out-DMA)
    are small; the middle runs at the full group size (bigger transfers =
    better DMA descriptor efficiency; 4096 cols A/B'd ~20us faster than
    2048)."""
    nt = -(-rc // TILE)                      # 37 tiles (last one short)
    mid = GCOLS // TILE
    inner = nt - 2
    sizes = [1] + [mid] * (inner // mid)
    if inner % mid:
        sizes.append(inner % mid)
    sizes.append(1)
    widths = []
    col = 0
    for s in sizes:
        w = min(s * TILE, rc - col)
        widths.append(w)
        col += w
    assert col == rc and all(w > 0 for w in widths)
    return widths

FT_ROWS = 432                # 128 res(c0-3) + 128 res(c4-7) + 128 atom(c0-7) + 48
OT_ROWS = 288                # 9 coef x 32 out channels
WT_COLS = 544                # 128 RA + 128 RB + 128 AA + 128 AB + 32 CC


def build_wt(w):
    """Stationary-weight image [128, 544] bf16. lhsT blocks are [Kin, Mout]."""
    w = np.asarray(w, np.float32)
    wt = np.zeros((128, WT_COLS), np.float32)
    for cl in range(4):
        # RA: res channels of coef cl -> out block cl
        wt[32 * cl:32 * cl + 32, 32 * cl:32 * cl + 32] = \
            w[L_OF_COEF[cl]][:, 16:48].T
        # RB: res channels of coef 4+cl
        wt[32 * cl:32 * cl + 32, 128 + 32 * cl:128 + 32 * cl + 32] = \
            w[L_OF_COEF[4 + cl]][:, 16:48].T
        # AA: atom channels of coef cl (K rows 0:64)
        wt[16 * cl:16 * cl + 16, 256 + 32 * cl:256 + 32 * cl + 32] = \
            w[L_OF_COEF[cl]][:, 0:16].T
        # AB: atom channels of coef 4+cl (K rows 64:128)
        wt[64 + 16 * cl:64 + 16 * cl + 16, 384 + 32 * cl:384 + 32 * cl + 32] = \
            w[L_OF_COEF[4 + cl]][:, 0:16].T
    # CC: coef 8, res (K 0:32) + atom (K 32:48) in one K=48 matmul
    wt[0:32, 512:544] = w[2][:, 16:48].T
    wt[32:48, 512:544] = w[2][:, 0:16].T
    return wt.astype(bfloat16)


def build_nc(rc=RC, n_rep=1, sb_bufs=3):
    """n_rep > 1 statically repeats the whole kernel body inside one NEFF
    (pure timing aid: slope between two n_rep values isolates kernel time
    from the per-dispatch overhead, which is ~70ms >> kernel time here)."""
    nc = bacc.Bacc()
    ft_d = nc.declare_dram_parameter("ft", [FT_ROWS * rc], BF16, isOutput=False)
    wt_d = nc.declare_dram_parameter("wt", [128, WT_COLS], BF16, isOutput=False)
    ot_d = nc.declare_dram_parameter("ot", [OT_ROWS * rc], BF16, isOutput=True)

    with TileContext(nc) as tc:
        with (
            tc.tile_pool(name="const", bufs=1) as cpool,
            tc.tile_pool(name="fin", bufs=sb_bufs) as fin_pool,
            tc.tile_pool(name="osb", bufs=sb_bufs) as osb_pool,
            tc.tile_pool(name="pMM", bufs=2, space="PSUM") as pMM_pool,
        ):
            wt_sb = cpool.tile([128, WT_COLS], BF16)
            nc.sync.dma_start(out=wt_sb[:], in_=wt_d[:])

            for _rep in range(n_rep):
                col = 0
                for W in _group_widths(rc):
                    # group-major tiled DRAM layout: each DMA below is one
                    # fully contiguous HBM span (A/B'd ~8us faster than the
                    # strided [rows, RC] layout)
                    fb = FT_ROWS * col
                    g0 = fin_pool.tile([128, GCOLS], BF16, tag="g0")
                    g1 = fin_pool.tile([128, GCOLS], BF16, tag="g1")
                    g2 = fin_pool.tile([128, GCOLS], BF16, tag="g2")
                    g3 = fin_pool.tile([48, GCOLS], BF16, tag="g3")
                    nc.sync.dma_start(
                        out=g0[:, 0:W],
                        in_=ft_d[fb:fb + 128 * W].rearrange("(p w) -> p w", w=W))
                    nc.sync.dma_start(
                        out=g1[:, 0:W],
                        in_=ft_d[fb + 128 * W:fb + 256 * W].rearrange(
                            "(p w) -> p w", w=W))
                    nc.sync.dma_start(
                        out=g2[:, 0:W],
                        in_=ft_d[fb + 256 * W:fb + 384 * W].rearrange(
                            "(p w) -> p w", w=W))
                    nc.sync.dma_start(
                        out=g3[:, 0:W],
                        in_=ft_d[fb + 384 * W:fb + 432 * W].rearrange(
                            "(p w) -> p w", w=W))

                    o0 = osb_pool.tile([128, GCOLS], BF16, tag="o0")
                    o1 = osb_pool.tile([128, GCOLS], BF16, tag="o1")
                    o2 = osb_pool.tile([32, GCOLS], BF16, tag="o2")

                    for t in range(-(-W // TILE)):
                        tw = min(TILE, W - TILE * t)
                        sl = slice(TILE * t, TILE * t + tw)
                        pA = pMM_pool.tile([128, TILE], F32, tag="pA")
                        pB = pMM_pool.tile([128, TILE], F32, tag="pB")
                        pC = pMM_pool.tile([32, TILE], F32, tag="pC")
                        nc.tensor.matmul(pA[:, 0:tw], wt_sb[0:128, 0:128],
                                         g0[:, sl], start=True, stop=False,
                                         skip_group_check=True,
                                         tile_position=(0, 0))
                        nc.tensor.matmul(pA[:, 0:tw], wt_sb[0:64, 256:384],
                                         g2[0:64, sl], start=False, stop=True,
                                         skip_group_check=True,
                                         tile_position=(0, 0))
                        nc.tensor.matmul(pB[:, 0:tw], wt_sb[0:128, 128:256],
                                         g1[:, sl], start=True, stop=False,
                                         skip_group_check=True,
                                         tile_position=(0, 0))
                        nc.tensor.matmul(pB[:, 0:tw], wt_sb[64:128, 384:512],
                                         g2[64:128, sl], start=False, stop=True,
                                         skip_group_check=True,
                                         tile_position=(64, 0))
                        nc.tensor.matmul(pC[:, 0:tw], wt_sb[0:48, 512:544],
                                         g3[0:48, sl], start=True, stop=True,
                                         skip_group_check=True,
                                         tile_position=(0, 0))
                        nc.scalar.copy(out=o0[:, sl], in_=pA[:, 0:tw])
                        nc.vector.tensor_copy(o1[:, sl], pB[:, 0:tw])
                        nc.vector.tensor_copy(o2[:, sl], pC[:, 0:tw])

                    # outputs go out on the second HWDGE ring (ACT) so the
                    # SP ring handles only the input stream
                    ob = OT_ROWS * col
                    nc.scalar.dma_start(
                        out=ot_d[ob:ob + 128 * W].rearrange("(p w) -> p w", w=W),
                        in_=o0[:, 0:W])
                    nc.scalar.dma_start(
                        out=ot_d[ob + 128 * W:ob + 256 * W].rearrange(
                            "(p w) -> p w", w=W),
                        in_=o1[:, 0:W])
                    nc.scalar.dma_start(
                        out=ot_d[ob + 256 * W:ob + 288 * W].rearrange(
                            "(p w) -> p w", w=W),
                        in_=o2[:, 0:W])
                    col += W
    nc.finalize()
    return nc


_NC_CACHE = {}


def _get_nc(rc=RC, n_rep=1):
    if (rc, n_rep) not in _NC_CACHE:
        _NC_CACHE[(rc, n_rep)] = build_nc(rc, n_rep)
    return _NC_CACHE[(rc, n_rep)]


def _make_in_maps(atom_agg, res_emb, w, b, backbone_idx, ca_res_idx):
    atom_agg = np.asarray(atom_agg)
    res_emb = np.asarray(res_emb)
    backbone_idx = np.asarray(backbone_idx)
    ca_res_idx = np.asarray(ca_res_idx)
    num_res = res_emb.shape[0]
    assert num_res == R_TOTAL, f"kernel compiled for {R_TOTAL} residues"

    wt = build_wt(w)
    E = res_emb.reshape(num_res, 288)
    A = atom_agg.reshape(atom_agg.shape[0], 144)

    ca_atom = backbone_idx.reshape(-1, 4)[:, 1]
    fast = (
        ca_atom.shape[0] == num_res
        and np.array_equal(ca_res_idx, np.arange(num_res, dtype=ca_res_idx.dtype))
        and np.array_equal(ca_atom, 4 * np.arange(num_res, dtype=ca_atom.dtype) + 1)
    )
    if fast:
        CA = A[1::4]                       # strided view [R, 144]
    else:
        CA = np.zeros((num_res, 144), np.float32)
        CA[ca_res_idx] = A[ca_atom]

    widths = _group_widths(RC)
    offs = np.cumsum([0] + widths[:-1]).tolist()
    in_maps = []
    for c in range(N_CORES):
        r0 = c * RS
        ft = np.empty((FT_ROWS, RC), bfloat16)
        Eb = E[r0:r0 + RC].astype(bfloat16)        # contiguous cast
        Cb = CA[r0:r0 + RC].astype(bfloat16)
        ft[0:256] = Eb.T[0:256]
        ft[256:384] = Cb.T[0:128]
        ft[384:416] = Eb.T[256:288]
        ft[416:432] = Cb.T[128:144]
        flat = np.concatenate(
            [np.ascontiguousarray(ft[:, o:o + w]).ravel()
             for o, w in zip(offs, widths)])
        in_maps.append({"ft": flat, "wt": wt})
    return in_maps


def _gather_out(results, b):
    widths = _group_widths(RC)
    offs = np.cumsum([0] + widths[:-1]).tolist()
    out = np.empty((R_TOTAL, 288), np.float32)
    for c in range(N_CORES):
        ot = np.asarray(results[c]["ot"]).ravel()
        for o, w in zip(offs, widths):
            blk = ot[OT_ROWS * o:OT_ROWS * (o + w)].reshape(OT_ROWS, w)
            out[c * RS + o:c * RS + o + w] = blk.T
    out[:, 0:32] += np.asarray(b, np.float32)[None, :]
    return out.reshape(R_TOTAL, NUM_COEF, NODE_C)


def _run(in_maps, trace=False, **kw):
    nc = _get_nc()
    return run_bass_kernel_spmd(nc, in_maps, core_ids=list(range(N_CORES)),
                                trace=trace, **kw)


def kernel(atom_agg, res_emb, w, b, backbone_idx, ca_res_idx):
    in_maps = _make_in_maps(atom_agg, res_emb, w, b, backbone_idx, ca_res_idx)
    res = _run(in_maps, trace=False)
    return _gather_out(res.results, b)


def kernel_profiled(atom_agg, res_emb, w, b, backbone_idx, ca_res_idx, **kw):
    """Same as kernel() but requests an NTFF trace; returns (out, results)."""
    in_maps = _make_in_maps(atom_agg, res_emb, w, b, backbone_idx, ca_res_idx)
    res = _run(in_maps, trace=True, **kw)
    return _gather_out(res.results, b), res


def build_null_nc(rc=RC):
    """Same I/O signature as build_nc but near-zero work — measures the
    per-call dispatch overhead so it can be subtracted."""
    nc = bacc.Bacc()
    ft_d = nc.declare_dram_parameter("ft", [FT_ROWS * rc], BF16, isOutput=False)
    nc.declare_dram_parameter("wt", [128, WT_COLS], BF16, isOutput=False)
    ot_d = nc.declare_dram_parameter("ot", [OT_ROWS * rc], BF16, isOutput=True)
    with TileContext(nc) as tc:
        with tc.tile_pool(name="t", bufs=1) as pool:
            tl = pool.tile([128, TILE], BF16)
            nc.sync.dma_start(
                out=tl[:],
                in_=ft_d[0:128 * TILE].rearrange("(p w) -> p w", w=TILE))
            nc.sync.dma_start(
                out=ot_d[0:128 * TILE].rearrange("(p w) -> p w", w=TILE),
                in_=tl[:])
    nc.finalize()
    return nc


def _timed_fn(nc, n_loop=1):
    """Build jitted 8-core executor that chains the NEFF n_loop times per
    call (outputs threaded back in as the pre-load of the output buffers),
    so per-exec time can be resolved under the ~70ms jax dispatch noise."""
    import jax
    from concourse import bass2jax as B

    B.install_neuronx_cc_hook()
    partition_name = nc.partition_id_tensor.name if nc.partition_id_tensor else None
    in_names, out_names, out_avals, zero_outs = [], [], [], []
    import concourse.mybir as mb
    for alloc in nc.m.functions[0].allocations:
        if not isinstance(alloc, mb.MemoryLocationSet):
            continue
        name = alloc.memorylocations[0].name
        if alloc.kind == "ExternalInput":
            if name != partition_name:
                in_names.append(name)
        elif alloc.kind == "ExternalOutput":
            shape = tuple(alloc.tensor_shape)
            dtype = mb.dt.np(alloc.dtype)
            out_avals.append(jax.core.ShapedArray(shape, dtype))
            out_names.append(name)
            zero_outs.append(np.zeros(shape, dtype))
    n_params = len(in_names)
    in_names = in_names + out_names
    if partition_name is not None:
        in_names.append(partition_name)

    assert n_loop == 1, "neuronx_cc_hook supports exactly one bass_exec per jit"

    def _body(*args):
        operands = list(args)
        if partition_name is not None:
            operands.append(B.partition_id_tensor())
        return tuple(B._bass_exec_p.bind(
            *operands,
            out_avals=tuple(out_avals),
            in_names=tuple(in_names),
            out_names=tuple(out_names),
            lowering_input_output_aliases=(),
            sim_require_finite=True,
            sim_require_nnan=True,
            nc=nc,
        ))

    mesh = B.Mesh(np.asarray(jax.devices()[:N_CORES]), ("core",))
    spec = B.PartitionSpec("core")
    fn = jax.jit(
        B.shard_map(_body, mesh=mesh,
                    in_specs=(spec,) * (n_params + len(out_names)),
                    out_specs=(spec,) * len(out_names), check_rep=False),
        keep_unused=True,
    )
    return fn, mesh, n_params, in_names, zero_outs


def kernel_timed(atom_agg, res_emb, w, b, backbone_idx, ca_res_idx,
                 cycles=40, n_lo=1, n_hi=101):
    """Returns (out, per_exec_seconds, info).

    The per-call wall time through the axon tunnel is ~70-110ms with
    multi-ms jitter, so a single NEFF execution (~0.1ms) is unmeasurable
    directly. Instead two NEFFs are built that statically repeat the
    IDENTICAL kernel body n_lo and n_hi times; their wall-time difference
    divided by (n_hi - n_lo) is the pure on-device per-execution time,
    independent of dispatch overhead and NEFF launch cost. Per cycle the
    call order is alternated (balanced) so position bias cancels; the
    estimate is the median of per-cycle paired differences."""
    import time

    import jax

    in_maps = _make_in_maps(atom_agg, res_emb, w, b, backbone_idx, ca_res_idx)

    def prep(nc):
        fn, mesh, n_params, in_names, zero_outs = _timed_fn(nc)
        spec = jax.sharding.NamedSharding(mesh, jax.sharding.PartitionSpec("core"))
        per_core = [[np.asarray(m[n]) for n in in_names[:n_params]] for m in in_maps]
        concat = [np.concatenate([per_core[c][i] for c in range(N_CORES)], 0)
                  for i in range(n_params)]
        concat += [np.zeros((N_CORES * z.shape[0], *z.shape[1:]), z.dtype)
                   for z in zero_outs]
        din = [jax.device_put(x, spec) for x in concat]
        outs = fn(*din)
        jax.block_until_ready(outs)  # compile + warm
        return fn, din, outs

    fn_lo, din_lo, outs = prep(_get_nc(n_rep=n_lo))
    fn_hi, din_hi, _ = prep(_get_nc(n_rep=n_hi))

    def timed(fn, din):
        t0 = time.perf_counter()
        jax.block_until_ready(fn(*din))
        return time.perf_counter() - t0

    diffs, ts_lo, ts_hi = [], [], []
    for cyc in range(cycles):
        if cyc % 2 == 0:
            tl = timed(fn_lo, din_lo)
            th = timed(fn_hi, din_hi)
        else:
            th = timed(fn_hi, din_hi)
            tl = timed(fn_lo, din_lo)
        ts_lo.append(tl)
        ts_hi.append(th)
        diffs.append((th - tl) / (n_hi - n_lo))

    diffs = np.array(diffs)
    per_exec = float(np.median(diffs))
    mad = float(np.median(np.abs(diffs - per_exec)))
    # cross-check with quantile slopes (robust when pair jitter is large)
    q_slopes = [(np.percentile(ts_hi, q) - np.percentile(ts_lo, q))
                / (n_hi - n_lo) for q in (10, 25, 50)]

    o = np.asarray(outs[0]).reshape(N_CORES, OT_ROWS * RC)
    results = [{"ot": o[c]} for c in range(N_CORES)]
    out_np = _gather_out(results, b)
    info = {"n": (n_lo, n_hi), "cycles": cycles,
            "paired_median_us": per_exec * 1e6,
            "paired_mad_us": mad * 1e6,
            "quantile_slopes_us": [s * 1e6 for s in q_slopes],
            "lo_ms_q": [float(np.percentile(np.array(ts_lo) * 1e3, q))
                        for q in (5, 25, 50)],
            "hi_ms_q": [float(np.percentile(np.array(ts_hi) * 1e3, q))
                        for q in (5, 25, 50)]}
    # Combine the four robust estimators; median guards against a single
    # corrupted statistic in a noisy tunnel window.
    est = float(np.median([per_exec] + q_slopes))
    return out_np, est, info


BUILDERS = {
    "v3_full": lambda: build_nc(),
    "null": lambda: build_null_nc(),
}



# revision 2
# speedup vs baseline: 1.0139x; 1.0139x over previous
"""Trainium2 Bass kernel: Atom2Residue (gnn_message_passing).

Math: out[n,c,o] = sum_i fuse[n,c,i] * w[l(c),o,i]  (+ b[o] at c==0)
where fuse[n,c,:] = concat(CA_atom[n,c,:16], res_emb[n,c,:32]), l(c)=floor(sqrt(c)).

Strategy (8 cores, data parallel over residues, no collectives):
  - Host pre-packs a channel-major fp8-E3M4 image ft[432, 18750] per core
    (residues on the free axis), so the device needs NO transposes:
      rows   0:128  res channels of coefs 0-3   (g0)
      rows 128:256  res channels of coefs 4-7   (g1)
      rows 256:384  atom channels of coefs 0-7  (g2)
      rows 384:432  res c8 (32) | atom c8 (16)  (g3)
    g0/g1/g2 are interleaved per-partition into ONE [128, 3W] DMA per
    column group; g3 is a second small [48, W] DMA.
  - Device: per 512-residue tile, 5 weight-stationary matmuls
    (block-diagonal per-coefficient bf16 weights x fp8 moving operand,
    PSUM f32 accumulation), PSUM -> SBUF bf16 copies, DMA out a
    transposed bf16 output image ot[288, 18750] (one [128, 2W] DMA for
    out-coefs 0-7 + one [32, W] for coef 8).
  - Host un-transposes the output, casts to f32, adds the l=0 bias.
  - fp8 E3M4 inputs (4 mantissa bits): rel-err 1.17e-2 < 2e-2 gate
    (e4m3 would fail at 2.3e-2). Weights stay bf16 (mixed-dtype matmul),
    outputs bf16.
  - HBM traffic/core: 8.1 MB in + 10.8 MB out = 18.9 MB -> 52.8 us
    roofline at 358 GB/s (vs 27 MB / 75 us for the all-bf16 variant).
  - DMAs batched in 4096-column groups; inputs on the SP HWDGE ring,
    outputs on the ACT ring; triple-buffered; first/last groups are a
    single 512-col tile to shrink the unoverlapped pipeline head/tail.
"""

import os
import sys

for _p in ("/opt/trn_rl_repo",):
    if os.path.isdir(_p) and _p not in sys.path:
        sys.path.insert(0, _p)

import numpy as np
from ml_dtypes import bfloat16, float8_e3m4

from concourse import bacc, bass, mybir  # noqa: F401
from concourse.bass_utils import run_bass_kernel_spmd
from concourse.tile import TileContext

F32 = mybir.dt.float32
BF16 = mybir.dt.bfloat16
FP8 = mybir.dt.float8e3

NUM_COEF, ATOM_C, NODE_C = 9, 16, 32
L_OF_COEF = np.floor(np.sqrt(np.arange(NUM_COEF))).astype(np.int64)

N_CORES = 8
R_TOTAL = 150_000
RS = R_TOTAL // N_CORES      # 18750 residues per core
TILE = 512                   # residues per matmul tile (PSUM bank = 512 f32)
RC = RS                      # exact columns per core (no padding)
GCOLS = 8 * TILE             # max columns per DMA group (4096)


def _group_widths(rc):
    """Column widths per DMA group. First/last groups are a single tile so
    the unoverlapped pipeline head (first in-DMA) and tail (last out-DMA)
    are small; the middle runs at the full group size."""
    nt = -(-rc // TILE)                      # 37 tiles (last one short)
    mid = GCOLS // TILE
    inner = nt - 2
    sizes = [1] + [mid] * (inner // mid)
    if inner % mid:
        sizes.append(inner % mid)
    sizes.append(1)
    widths = []
    col = 0
    for s in sizes:
        w = min(s * TILE, rc - col)
        widths.append(w)
        col += w
    assert col == rc and all(w > 0 for w in widths)
    return widths

FT_ROWS = 432                # 128 res(c0-3) + 128 res(c4-7) + 128 atom(c0-7) + 48
OT_ROWS = 288                # 9 coef x 32 out channels
WT_COLS = 544                # 128 RA + 128 RB + 128 AA + 128 AB + 32 CC


def build_wt(w):
    """Stationary-weight image [128, 544] bf16. lhsT blocks are [Kin, Mout]."""
    w = np.asarray(w, np.float32)
    wt = np.zeros((128, WT_COLS), np.float32)
    for cl in range(4):
        # RA: res channels of coef cl -> out block cl
        wt[32 * cl:32 * cl + 32, 32 * cl:32 * cl + 32] = \
            w[L_OF_COEF[cl]][:, 16:48].T
        # RB: res channels of coef 4+cl
        wt[32 * cl:32 * cl + 32, 128 + 32 * cl:128 + 32 * cl + 32] = \
            w[L_OF_COEF[4 + cl]][:, 16:48].T
        # AA: atom channels of coef cl (K rows 0:64)
        wt[16 * cl:16 * cl + 16, 256 + 32 * cl:256 + 32 * cl + 32] = \
            w[L_OF_COEF[cl]][:, 0:16].T
        # AB: atom channels of coef 4+cl (K rows 64:128)
        wt[64 + 16 * cl:64 + 16 * cl + 16, 384 + 32 * cl:384 + 32 * cl + 32] = \
            w[L_OF_COEF[4 + cl]][:, 0:16].T
    # CC: coef 8, res (K 0:32) + atom (K 32:48) in one K=48 matmul
    wt[0:32, 512:544] = w[2][:, 16:48].T
    wt[32:48, 512:544] = w[2][:, 0:16].T
    return wt.astype(bfloat16)


def build_nc(rc=RC, n_rep=1, sb_bufs=3):
    """n_rep > 1 statically repeats the whole kernel body inside one NEFF
    (pure timing aid: slope between two n_rep values isolates kernel time
    from the per-dispatch overhead, which is ~70ms >> kernel time here)."""
    nc = bacc.Bacc()
    ft_d = nc.declare_dram_parameter("ft", [FT_ROWS * rc], FP8, isOutput=False)
    wt_d = nc.declare_dram_parameter("wt", [128, WT_COLS], BF16, isOutput=False)
    ot_d = nc.declare_dram_parameter("ot", [OT_ROWS * rc], BF16, isOutput=True)

    with TileContext(nc) as tc:
        with (
            tc.tile_pool(name="const", bufs=1) as cpool,
            tc.tile_pool(name="fin", bufs=sb_bufs) as fin_pool,
            tc.tile_pool(name="osb", bufs=sb_bufs) as osb_pool,
            tc.tile_pool(name="pMM", bufs=2, space="PSUM") as pMM_pool,
        ):
            wt_sb = cpool.tile([128, WT_COLS], BF16)
            nc.sync.dma_start(out=wt_sb[:], in_=wt_d[:])

            for _rep in range(n_rep):
                col = 0
                for W in _group_widths(rc):
                    # group-major tiled DRAM layout: each DMA below is one
                    # fully contiguous HBM span
                    fb = FT_ROWS * col
                    g012 = fin_pool.tile([128, 3 * GCOLS], FP8, tag="g012")
                    g3 = fin_pool.tile([48, GCOLS], FP8, tag="g3")
                    nc.sync.dma_start(
                        out=g012[:, 0:3 * W],
                        in_=ft_d[fb:fb + 384 * W].rearrange(
                            "(p w) -> p w", w=3 * W))
                    nc.sync.dma_start(
                        out=g3[:, 0:W],
                        in_=ft_d[fb + 384 * W:fb + 432 * W].rearrange(
                            "(p w) -> p w", w=W))

                    o01 = osb_pool.tile([128, 2 * GCOLS], BF16, tag="o01")
                    o2 = osb_pool.tile([32, GCOLS], BF16, tag="o2")

                    for t in range(-(-W // TILE)):
                        tw = min(TILE, W - TILE * t)
                        c0 = TILE * t
                        pA = pMM_pool.tile([128, TILE], F32, tag="pA")
                        pB = pMM_pool.tile([128, TILE], F32, tag="pB")
                        pC = pMM_pool.tile([32, TILE], F32, tag="pC")
                        nc.tensor.matmul(pA[:, 0:tw], wt_sb[0:128, 0:128],
                                         g012[:, c0:c0 + tw],
                                         start=True, stop=False,
                                         skip_group_check=True,
                                         tile_position=(0, 0))
                        nc.tensor.matmul(pA[:, 0:tw], wt_sb[0:64, 256:384],
                                         g012[0:64, 2 * W + c0:2 * W + c0 + tw],
                                         start=False, stop=True,
                                         skip_group_check=True,
                                         tile_position=(0, 0))
                        nc.tensor.matmul(pB[:, 0:tw], wt_sb[0:128, 128:256],
                                         g012[:, W + c0:W + c0 + tw],
                                         start=True, stop=False,
                                         skip_group_check=True,
                                         tile_position=(0, 0))
                        nc.tensor.matmul(pB[:, 0:tw], wt_sb[64:128, 384:512],
                                         g012[64:128, 2 * W + c0:2 * W + c0 + tw],
                                         start=False, stop=True,
                                         skip_group_check=True,
                                         tile_position=(64, 0))
                        nc.tensor.matmul(pC[:, 0:tw], wt_sb[0:48, 512:544],
                                         g3[0:48, c0:c0 + tw],
                                         start=True, stop=True,
                                         skip_group_check=True,
                                         tile_position=(0, 0))
                        nc.scalar.copy(out=o01[:, c0:c0 + tw], in_=pA[:, 0:tw])
                        nc.vector.tensor_copy(o01[:, W + c0:W + c0 + tw],
                                              pB[:, 0:tw])
                        nc.vector.tensor_copy(o2[:, c0:c0 + tw], pC[:, 0:tw])

                    # outputs go out on the second HWDGE ring (ACT) so the
                    # SP ring handles only the input stream
                    ob = OT_ROWS * col
                    nc.scalar.dma_start(
                        out=ot_d[ob:ob + 256 * W].rearrange(
                            "(p w) -> p w", w=2 * W),
                        in_=o01[:, 0:2 * W])
                    nc.scalar.dma_start(
                        out=ot_d[ob + 256 * W:ob + 288 * W].rearrange(
                            "(p w) -> p w", w=W),
                        in_=o2[:, 0:W])
                    col += W
    nc.finalize()
    return nc


_NC_CACHE = {}


def _get_nc(rc=RC, n_rep=1):
    if (rc, n_rep) not in _NC_CACHE:
        _NC_CACHE[(rc, n_rep)] = build_nc(rc, n_rep)
    return _NC_CACHE[(rc, n_rep)]


def _make_in_maps(atom_agg, res_emb, w, b, backbone_idx, ca_res_idx):
    atom_agg = np.asarray(atom_agg)
    res_emb = np.asarray(res_emb)
    backbone_idx = np.asarray(backbone_idx)
    ca_res_idx = np.asarray(ca_res_idx)
    num_res = res_emb.shape[0]
    assert num_res == R_TOTAL, f"kernel compiled for {R_TOTAL} residues"

    wt = build_wt(w)
    E = res_emb.reshape(num_res, 288)
    A = atom_agg.reshape(atom_agg.shape[0], 144)

    ca_atom = backbone_idx.reshape(-1, 4)[:, 1]
    fast = (
        ca_atom.shape[0] == num_res
        and np.array_equal(ca_res_idx, np.arange(num_res, dtype=ca_res_idx.dtype))
        and np.array_equal(ca_atom, 4 * np.arange(num_res, dtype=ca_atom.dtype) + 1)
    )
    if fast:
        CA = A[1::4]                       # strided view [R, 144]
    else:
        CA = np.zeros((num_res, 144), np.float32)
        CA[ca_res_idx] = A[ca_atom]

    widths = _group_widths(RC)
    offs = np.cumsum([0] + widths[:-1]).tolist()
    in_maps = []
    for c in range(N_CORES):
        r0 = c * RS
        ft = np.empty((FT_ROWS, RC), float8_e3m4)
        Eb = E[r0:r0 + RC].astype(float8_e3m4)     # contiguous cast
        Cb = CA[r0:r0 + RC].astype(float8_e3m4)
        ft[0:256] = Eb.T[0:256]
        ft[256:384] = Cb.T[0:128]
        ft[384:416] = Eb.T[256:288]
        ft[416:432] = Cb.T[128:144]
        # per-group: [128, 3, w] interleave of g0/g1/g2, then [48, w] g3
        blocks = []
        for o, wdt in zip(offs, widths):
            m = np.ascontiguousarray(
                ft[0:384, o:o + wdt].reshape(3, 128, wdt).transpose(1, 0, 2))
            blocks.append(m.ravel())
            blocks.append(np.ascontiguousarray(ft[384:432, o:o + wdt]).ravel())
        in_maps.append({"ft": np.concatenate(blocks), "wt": wt})
    return in_maps


def _gather_out(results, b):
    widths = _group_widths(RC)
    offs = np.cumsum([0] + widths[:-1]).tolist()
    out = np.empty((R_TOTAL, 288), np.float32)
    for c in range(N_CORES):
        ot = np.asarray(results[c]["ot"]).ravel()
        for o, w in zip(offs, widths):
            blk01 = ot[OT_ROWS * o:OT_ROWS * o + 256 * w].reshape(128, 2, w)
            blk2 = ot[OT_ROWS * o + 256 * w:OT_ROWS * (o + w)].reshape(32, w)
            r0 = c * RS + o
            out[r0:r0 + w, 0:128] = blk01[:, 0].T
            out[r0:r0 + w, 128:256] = blk01[:, 1].T
            out[r0:r0 + w, 256:288] = blk2.T
    out[:, 0:32] += np.asarray(b, np.float32)[None, :]
    return out.reshape(R_TOTAL, NUM_COEF, NODE_C)


def _run(in_maps, trace=False, **kw):
    nc = _get_nc()
    return run_bass_kernel_spmd(nc, in_maps, core_ids=list(range(N_CORES)),
                                trace=trace, **kw)


def kernel(atom_agg, res_emb, w, b, backbone_idx, ca_res_idx):
    in_maps = _make_in_maps(atom_agg, res_emb, w, b, backbone_idx, ca_res_idx)
    res = _run(in_maps, trace=False)
    return _gather_out(res.results, b)


def kernel_profiled(atom_agg, res_emb, w, b, backbone_idx, ca_res_idx, **kw):
    """Same as kernel() but requests an NTFF trace; returns (out, results)."""
    in_maps = _make_in_maps(atom_agg, res_emb, w, b, backbone_idx, ca_res_idx)
    res = _run(in_maps, trace=True, **kw)
    return _gather_out(res.results, b), res


def _timed_fn(nc, n_loop=1):
    """Build jitted 8-core executor that chains the NEFF n_loop times per
    call (outputs threaded back in as the pre-load of the output buffers),
    so per-exec time can be resolved under the ~70ms jax dispatch noise."""
    import jax
    from concourse import bass2jax as B

    B.install_neuronx_cc_hook()
    partition_name = nc.partition_id_tensor.name if nc.partition_id_tensor else None
    in_names, out_names, out_avals, zero_outs = [], [], [], []
    import concourse.mybir as mb
    for alloc in nc.m.functions[0].allocations:
        if not isinstance(alloc, mb.MemoryLocationSet):
            continue
        name = alloc.memorylocations[0].name
        if alloc.kind == "ExternalInput":
            if name != partition_name:
                in_names.append(name)
        elif alloc.kind == "ExternalOutput":
            shape = tuple(alloc.tensor_shape)
            dtype = mb.dt.np(alloc.dtype)
            out_avals.append(jax.core.ShapedArray(shape, dtype))
            out_names.append(name)
            zero_outs.append(np.zeros(shape, dtype))
    n_params = len(in_names)
    in_names = in_names + out_names
    if partition_name is not None:
        in_names.append(partition_name)

    assert n_loop == 1, "neuronx_cc_hook supports exactly one bass_exec per jit"

    def _body(*args):
        operands = list(args)
        if partition_name is not None:
            operands.append(B.partition_id_tensor())
        return tuple(B._bass_exec_p.bind(
            *operands,
            out_avals=tuple(out_avals),
            in_names=tuple(in_names),
            out_names=tuple(out_names),
            lowering_input_output_aliases=(),
            sim_require_finite=True,
            sim_require_nnan=True,
            nc=nc,
        ))

    mesh = B.Mesh(np.asarray(jax.devices()[:N_CORES]), ("core",))
    spec = B.PartitionSpec("core")
    fn = jax.jit(
        B.shard_map(_body, mesh=mesh,
                    in_specs=(spec,) * (n_params + len(out_names)),
                    out_specs=(spec,) * len(out_names), check_rep=False),
        keep_unused=True,
    )
    return fn, mesh, n_params, in_names, zero_outs


def kernel_timed(atom_agg, res_emb, w, b, backbone_idx, ca_res_idx,
                 cycles=40, n_lo=1, n_hi=101):
    """Returns (out, per_exec_seconds, info).

    The per-call wall time through the axon tunnel is ~70-110ms with
    multi-ms jitter, so a single NEFF execution (~0.1ms) is unmeasurable
    directly. Instead two NEFFs are built that statically repeat the
    IDENTICAL kernel body n_lo and n_hi times; their wall-time difference
    divided by (n_hi - n_lo) is the pure on-device per-execution time,
    independent of dispatch overhead and NEFF launch cost. Per cycle the
    call order is alternated (balanced) so position bias cancels; the
    estimate is the median of per-cycle paired differences."""
    import time

    import jax

    in_maps = _make_in_maps(atom_agg, res_emb, w, b, backbone_idx, ca_res_idx)

    def prep(nc):
        fn, mesh, n_params, in_names, zero_outs = _timed_fn(nc)
        spec = jax.sharding.NamedSharding(mesh, jax.sharding.PartitionSpec("core"))
        per_core = [[np.asarray(m[n]) for n in in_names[:n_params]] for m in in_maps]
        concat = [np.concatenate([per_core[c][i] for c in range(N_CORES)], 0)
                  for i in range(n_params)]
        concat += [np.zeros((N_CORES * z.shape[0], *z.shape[1:]), z.dtype)
                   for z in zero_outs]
        din = [jax.device_put(x, spec) for x in concat]
        outs = fn(*din)
        jax.block_until_ready(outs)  # compile + warm
        return fn, din, outs

    fn_lo, din_lo, outs = prep(_get_nc(n_rep=n_lo))
    fn_hi, din_hi, _ = prep(_get_nc(n_rep=n_hi))

    def timed(fn, din):
        t0 = time.perf_counter()
        jax.block_until_ready(fn(*din))
        return time.perf_counter() - t0

    diffs, ts_lo, ts_hi = [], [], []
    for cyc in range(cycles):
        if cyc % 2 == 0:
            tl = timed(fn_lo, din_lo)
            th = timed(fn_hi, din_hi)
        else:
            th = timed(fn_hi, din_hi)
            tl = timed(fn_lo, din_lo)
        ts_lo.append(tl)
        ts_hi.append(th)
        diffs.append((th - tl) / (n_hi - n_lo))

    diffs = np.array(diffs)
    per_exec = float(np.median(diffs))
    mad = float(np.median(np.abs(diffs - per_exec)))
    # cross-check with quantile slopes (robust when pair jitter is large)
    q_slopes = [(np.percentile(ts_hi, q) - np.percentile(ts_lo, q))
                / (n_hi - n_lo) for q in (10, 25, 50)]

    o = np.asarray(outs[0]).reshape(N_CORES, OT_ROWS * RC)
    results = [{"ot": o[c]} for c in range(N_CORES)]
    out_np = _gather_out(results, b)
    info = {"n": (n_lo, n_hi), "cycles": cycles,
            "paired_median_us": per_exec * 1e6,
            "paired_mad_us": mad * 1e6,
            "quantile_slopes_us": [s * 1e6 for s in q_slopes],
            "lo_ms_q": [float(np.percentile(np.array(ts_lo) * 1e3, q))
                        for q in (5, 25, 50)],
            "hi_ms_q": [float(np.percentile(np.array(ts_hi) * 1e3, q))
                        for q in (5, 25, 50)]}
    # Combine the four robust estimators; median guards against a single
    # corrupted statistic in a noisy tunnel window.
    est = float(np.median([per_exec] + q_slopes))
    return out_np, est, info


BUILDERS = {
    "v4_fp8": lambda: build_nc(),
}


# revision 3
# speedup vs baseline: 1.4817x; 1.4615x over previous
"""Trainium2 Bass kernel: Atom2Residue (gnn_message_passing).

Math: out[n,c,o] = sum_i fuse[n,c,i] * w[l(c),o,i]  (+ b[o] at c==0)
where fuse[n,c,:] = concat(CA_atom[n,c,:16], res_emb[n,c,:32]), l(c)=floor(sqrt(c)).

Strategy (8 cores, data parallel over residues, no collectives):
  - Host pre-packs a channel-major fp8-E3M4 image ft[432, 18750] per core
    (residues on the free axis; E3M4's 4 mantissa bits keep rel-err at
    1.17e-2 < 2e-2 gate, e4m3 would fail at 2.3e-2), so the device needs
    NO transposes:
      rows   0:128  res channels of coefs 0-3   (g0)
      rows 128:256  res channels of coefs 4-7   (g1)
      rows 256:384  atom channels of coefs 0-7  (g2)
      rows 384:432  res c8 (32) | atom c8 (16)  (g3)
    g0/g1/g2 are interleaved per-partition into ONE [128, 3W] DMA per
    column group; g3 is a second small [48, W] DMA.
  - Device compute is PE-bound (measured, not DMA-bound: the two DMA
    streams alone run 42 us/rep vs 81 us for the naive matmul order), so
    matmuls run WEIGHT-OUTERMOST over chunks of 3 residue-tiles: each of
    the 5 block-diagonal stationary operands (bf16, mixed-dtype matmul
    with the fp8 moving operand) is loaded once per chunk and streams 3
    N=512 matmuls, amortizing the ~160 ns LDWEIGHTS+drain per switch
    (measured: 81 us -> 55 us for the matmul stream).
  - PSUM budget (8 banks): pA0-2 + pB0-2 single-buffered + pC [128,512]
    double-buffered, where chunk tile j's coef-8 output lives at pC
    partitions 32j (one bank for the whole chunk).
  - PSUM -> SBUF bf16 copies split ACT (pA) / DVE (pB, pC); outputs DMA
    out on the ACT ring as a [128, 2W] o01 image + [96, 512*nchunks] o2
    image per group; inputs on the SP ring; triple-buffered.
  - Host un-transposes the output, casts to f32, adds the l=0 bias.
  - HBM traffic/core: 8.1 MB in (fp8) + 10.9 MB out (bf16).
"""

import os
import sys

for _p in ("/opt/trn_rl_repo",):
    if os.path.isdir(_p) and _p not in sys.path:
        sys.path.insert(0, _p)

import numpy as np
from ml_dtypes import bfloat16, float8_e3m4

from concourse import bacc, bass, mybir  # noqa: F401
from concourse.bass_utils import run_bass_kernel_spmd
from concourse.tile import TileContext

F32 = mybir.dt.float32
BF16 = mybir.dt.bfloat16
FP8 = mybir.dt.float8e3

NUM_COEF, ATOM_C, NODE_C = 9, 16, 32
L_OF_COEF = np.floor(np.sqrt(np.arange(NUM_COEF))).astype(np.int64)

N_CORES = 8
R_TOTAL = 150_000
RS = R_TOTAL // N_CORES      # 18750 residues per core
TILE = 512                   # residues per matmul tile (PSUM bank = 512 f32)
RC = RS                      # exact columns per core (no padding)
CH = 3                       # residue-tiles per weight-reuse chunk (PSUM cap)

FT_ROWS = 432                # 128 res(c0-3) + 128 res(c4-7) + 128 atom(c0-7) + 48
WT_COLS = 544                # 128 RA + 128 RB + 128 AA + 128 AB + 32 CC


def _layout(rc=RC):
    """Groups of chunks: first group 1 chunk (small pipeline head), rest 2
    chunks per group. Returns list of groups; each group is a list of chunk
    tile-width lists, e.g. [[512,512,512],[512,512,318]]."""
    nt = -(-rc // TILE)
    tiles = [min(TILE, rc - TILE * t) for t in range(nt)]
    chunks = [tiles[i:i + CH] for i in range(0, nt, CH)]
    groups = [[chunks[0]]]
    i = 1
    while i < len(chunks):
        groups.append(chunks[i:i + 2])
        i += 2
    return groups


def _group_dims(g):
    """(total width, nchunks) of a group."""
    return sum(sum(c) for c in g), len(g)


def build_wt(w):
    """Stationary-weight image [128, 544] bf16. lhsT blocks are [Kin, Mout]."""
    w = np.asarray(w, np.float32)
    wt = np.zeros((128, WT_COLS), np.float32)
    for cl in range(4):
        # RA: res channels of coef cl -> out block cl
        wt[32 * cl:32 * cl + 32, 32 * cl:32 * cl + 32] = \
            w[L_OF_COEF[cl]][:, 16:48].T
        # RB: res channels of coef 4+cl
        wt[32 * cl:32 * cl + 32, 128 + 32 * cl:128 + 32 * cl + 32] = \
            w[L_OF_COEF[4 + cl]][:, 16:48].T
        # AA: atom channels of coef cl (K rows 0:64)
        wt[16 * cl:16 * cl + 16, 256 + 32 * cl:256 + 32 * cl + 32] = \
            w[L_OF_COEF[cl]][:, 0:16].T
        # AB: atom channels of coef 4+cl (K rows 64:128)
        wt[64 + 16 * cl:64 + 16 * cl + 16, 384 + 32 * cl:384 + 32 * cl + 32] = \
            w[L_OF_COEF[4 + cl]][:, 0:16].T
    # CC: coef 8, res (K 0:32) + atom (K 32:48) in one K=48 matmul
    wt[0:32, 512:544] = w[2][:, 16:48].T
    wt[32:48, 512:544] = w[2][:, 0:16].T
    return wt.astype(bfloat16)


def build_nc(rc=RC, n_rep=1, sb_bufs=3):
    """n_rep > 1 statically repeats the whole kernel body inside one NEFF
    (pure timing aid: slope between two n_rep values isolates kernel time
    from the per-dispatch overhead, which is ~70ms >> kernel time here)."""
    groups = _layout(rc)
    nc = bacc.Bacc()
    ft_total = FT_ROWS * rc
    ot_total = 256 * rc + 96 * TILE * sum(len(g) for g in groups)
    ft_d = nc.declare_dram_parameter("ft", [ft_total], FP8, isOutput=False)
    wt_d = nc.declare_dram_parameter("wt", [128, WT_COLS], BF16, isOutput=False)
    ot_d = nc.declare_dram_parameter("ot", [ot_total], BF16, isOutput=True)

    with TileContext(nc) as tc:
        with (
            tc.tile_pool(name="const", bufs=1) as cpool,
            tc.tile_pool(name="fin", bufs=sb_bufs) as fin_pool,
            tc.tile_pool(name="osb", bufs=sb_bufs) as osb_pool,
            tc.tile_pool(name="pAB", bufs=1, space="PSUM") as pAB_pool,
            tc.tile_pool(name="pCC", bufs=2, space="PSUM") as pCC_pool,
        ):
            wt_sb = cpool.tile([128, WT_COLS], BF16)
            nc.sync.dma_start(out=wt_sb[:], in_=wt_d[:])

            GW = max(_group_dims(g)[0] for g in groups)
            GNC = max(_group_dims(g)[1] for g in groups)

            for _rep in range(n_rep):
                fb = 0
                ob = 0
                for g in groups:
                    W, nch = _group_dims(g)
                    g012 = fin_pool.tile([128, 3 * GW], FP8, tag="g012")
                    g3 = fin_pool.tile([48, GW], FP8, tag="g3")
                    nc.sync.dma_start(
                        out=g012[:, 0:3 * W],
                        in_=ft_d[fb:fb + 384 * W].rearrange(
                            "(p w) -> p w", w=3 * W))
                    nc.sync.dma_start(
                        out=g3[:, 0:W],
                        in_=ft_d[fb + 384 * W:fb + 432 * W].rearrange(
                            "(p w) -> p w", w=W))
                    fb += 432 * W

                    o01 = osb_pool.tile([128, 2 * GW], BF16, tag="o01")
                    o2 = osb_pool.tile([96, TILE * GNC], BF16, tag="o2")

                    col = 0
                    for k, chunk in enumerate(g):
                        ch = len(chunk)
                        cols = [col + TILE * j for j in range(ch)]
                        pA = [pAB_pool.tile([128, TILE], F32, tag=f"pA{j}",
                                            name=f"pA{j}") for j in range(ch)]
                        pB = [pAB_pool.tile([128, TILE], F32, tag=f"pB{j}",
                                            name=f"pB{j}") for j in range(ch)]
                        pC = pCC_pool.tile([128, TILE], F32, tag="pC")
                        # pass RA: res coefs 0-3 (LDW once per chunk)
                        for j, tw in enumerate(chunk):
                            nc.tensor.matmul(
                                pA[j][:, 0:tw], wt_sb[0:128, 0:128],
                                g012[:, cols[j]:cols[j] + tw],
                                start=True, stop=False,
                                skip_group_check=True, tile_position=(0, 0))
                        # pass AA: atom coefs 0-3 accumulate
                        for j, tw in enumerate(chunk):
                            nc.tensor.matmul(
                                pA[j][:, 0:tw], wt_sb[0:64, 256:384],
                                g012[0:64, 2 * W + cols[j]:2 * W + cols[j] + tw],
                                start=False, stop=True,
                                skip_group_check=True, tile_position=(0, 0))
                        # pass RB: res coefs 4-7
                        for j, tw in enumerate(chunk):
                            nc.tensor.matmul(
                                pB[j][:, 0:tw], wt_sb[0:128, 128:256],
                                g012[:, W + cols[j]:W + cols[j] + tw],
                                start=True, stop=False,
                                skip_group_check=True, tile_position=(0, 0))
                        # pass AB: atom coefs 4-7 accumulate
                        for j, tw in enumerate(chunk):
                            nc.tensor.matmul(
                                pB[j][:, 0:tw], wt_sb[64:128, 384:512],
                                g012[64:128,
                                     2 * W + cols[j]:2 * W + cols[j] + tw],
                                start=False, stop=True,
                                skip_group_check=True, tile_position=(64, 0))
                        # pass CC: coef 8; chunk tile j -> pC partitions 32j
                        for j, tw in enumerate(chunk):
                            nc.tensor.matmul(
                                pC[32 * j:32 * j + 32, 0:tw],
                                wt_sb[0:48, 512:544],
                                g3[0:48, cols[j]:cols[j] + tw],
                                start=True, stop=True,
                                skip_group_check=True,
                                tile_position=(0, 32 * j))
                        # PSUM evacuation: pA on ACT, pB + pC on DVE
                        for j, tw in enumerate(chunk):
                            nc.scalar.copy(out=o01[:, cols[j]:cols[j] + tw],
                                           in_=pA[j][:, 0:tw])
                            nc.vector.tensor_copy(
                                o01[:, W + cols[j]:W + cols[j] + tw],
                                pB[j][:, 0:tw])
                        nc.vector.tensor_copy(
                            o2[0:32 * ch, TILE * k:TILE * k + TILE],
                            pC[0:32 * ch, :])
                        col += sum(chunk)

                    # outputs on the second HWDGE ring (ACT); SP carries
                    # only the input stream
                    nc.scalar.dma_start(
                        out=ot_d[ob:ob + 256 * W].rearrange(
                            "(p w) -> p w", w=2 * W),
                        in_=o01[:, 0:2 * W])
                    nc.scalar.dma_start(
                        out=ot_d[ob + 256 * W:ob + 256 * W + 96 * TILE * nch]
                        .rearrange("(p w) -> p w", w=TILE * nch),
                        in_=o2[:, 0:TILE * nch])
                    ob += 256 * W + 96 * TILE * nch
    nc.finalize()
    return nc


_NC_CACHE = {}


def _get_nc(rc=RC, n_rep=1):
    if (rc, n_rep) not in _NC_CACHE:
        _NC_CACHE[(rc, n_rep)] = build_nc(rc, n_rep)
    return _NC_CACHE[(rc, n_rep)]


def _make_in_maps(atom_agg, res_emb, w, b, backbone_idx, ca_res_idx):
    atom_agg = np.asarray(atom_agg)
    res_emb = np.asarray(res_emb)
    backbone_idx = np.asarray(backbone_idx)
    ca_res_idx = np.asarray(ca_res_idx)
    num_res = res_emb.shape[0]
    assert num_res == R_TOTAL, f"kernel compiled for {R_TOTAL} residues"

    wt = build_wt(w)
    E = res_emb.reshape(num_res, 288)
    A = atom_agg.reshape(atom_agg.shape[0], 144)

    ca_atom = backbone_idx.reshape(-1, 4)[:, 1]
    fast = (
        ca_atom.shape[0] == num_res
        and np.array_equal(ca_res_idx, np.arange(num_res, dtype=ca_res_idx.dtype))
        and np.array_equal(ca_atom, 4 * np.arange(num_res, dtype=ca_atom.dtype) + 1)
    )
    if fast:
        CA = A[1::4]                       # strided view [R, 144]
    else:
        CA = np.zeros((num_res, 144), np.float32)
        CA[ca_res_idx] = A[ca_atom]

    groups = _layout(RC)
    in_maps = []
    for c in range(N_CORES):
        r0 = c * RS
        ft = np.empty((FT_ROWS, RC), float8_e3m4)
        Eb = E[r0:r0 + RC].astype(float8_e3m4)     # contiguous cast
        Cb = CA[r0:r0 + RC].astype(float8_e3m4)
        ft[0:256] = Eb.T[0:256]
        ft[256:384] = Cb.T[0:128]
        ft[384:416] = Eb.T[256:288]
        ft[416:432] = Cb.T[128:144]
        # per-group: [128, 3, W] interleave of g0/g1/g2, then [48, W] g3
        blocks = []
        o = 0
        for g in groups:
            W, _ = _group_dims(g)
            m = np.ascontiguousarray(
                ft[0:384, o:o + W].reshape(3, 128, W).transpose(1, 0, 2))
            blocks.append(m.ravel())
            blocks.append(np.ascontiguousarray(ft[384:432, o:o + W]).ravel())
            o += W
        in_maps.append({"ft": np.concatenate(blocks), "wt": wt})
    return in_maps


def _gather_out(results, b):
    groups = _layout(RC)
    out = np.empty((R_TOTAL, 288), np.float32)
    for c in range(N_CORES):
        ot = np.asarray(results[c]["ot"]).ravel()
        ob = 0
        o = 0
        for g in groups:
            W, nch = _group_dims(g)
            blk01 = ot[ob:ob + 256 * W].reshape(128, 2, W)
            blk2 = ot[ob + 256 * W:ob + 256 * W + 96 * TILE * nch].reshape(
                96, TILE * nch)
            r0 = c * RS + o
            out[r0:r0 + W, 0:128] = blk01[:, 0].T
            out[r0:r0 + W, 128:256] = blk01[:, 1].T
            col = 0
            for k, chunk in enumerate(g):
                for j, tw in enumerate(chunk):
                    out[r0 + col:r0 + col + tw, 256:288] = \
                        blk2[32 * j:32 * j + 32,
                             TILE * k:TILE * k + tw].T
                    col += tw
            ob += 256 * W + 96 * TILE * nch
            o += W
    out[:, 0:32] += np.asarray(b, np.float32)[None, :]
    return out.reshape(R_TOTAL, NUM_COEF, NODE_C)


def _run(in_maps, trace=False, **kw):
    nc = _get_nc()
    return run_bass_kernel_spmd(nc, in_maps, core_ids=list(range(N_CORES)),
                                trace=trace, **kw)


def kernel(atom_agg, res_emb, w, b, backbone_idx, ca_res_idx):
    in_maps = _make_in_maps(atom_agg, res_emb, w, b, backbone_idx, ca_res_idx)
    res = _run(in_maps, trace=False)
    return _gather_out(res.results, b)


def _timed_fn(nc, n_loop=1):
    """Build jitted 8-core executor (single NEFF exec per call)."""
    import jax
    from concourse import bass2jax as B

    B.install_neuronx_cc_hook()
    partition_name = nc.partition_id_tensor.name if nc.partition_id_tensor else None
    in_names, out_names, out_avals, zero_outs = [], [], [], []
    import concourse.mybir as mb
    for alloc in nc.m.functions[0].allocations:
        if not isinstance(alloc, mb.MemoryLocationSet):
            continue
        name = alloc.memorylocations[0].name
        if alloc.kind == "ExternalInput":
            if name != partition_name:
                in_names.append(name)
        elif alloc.kind == "ExternalOutput":
            shape = tuple(alloc.tensor_shape)
            dtype = mb.dt.np(alloc.dtype)
            out_avals.append(jax.core.ShapedArray(shape, dtype))
            out_names.append(name)
            zero_outs.append(np.zeros(shape, dtype))
    n_params = len(in_names)
    in_names = in_names + out_names
    if partition_name is not None:
        in_names.append(partition_name)

    assert n_loop == 1, "neuronx_cc_hook supports exactly one bass_exec per jit"

    def _body(*args):
        operands = list(args)
        if partition_name is not None:
            operands.append(B.partition_id_tensor())
        return tuple(B._bass_exec_p.bind(
            *operands,
            out_avals=tuple(out_avals),
            in_names=tuple(in_names),
            out_names=tuple(out_names),
            lowering_input_output_aliases=(),
            sim_require_finite=True,
            sim_require_nnan=True,
            nc=nc,
        ))

    mesh = B.Mesh(np.asarray(jax.devices()[:N_CORES]), ("core",))
    spec = B.PartitionSpec("core")
    fn = jax.jit(
        B.shard_map(_body, mesh=mesh,
                    in_specs=(spec,) * (n_params + len(out_names)),
                    out_specs=(spec,) * len(out_names), check_rep=False),
        keep_unused=True,
    )
    return fn, mesh, n_params, in_names, zero_outs


def kernel_timed(atom_agg, res_emb, w, b, backbone_idx, ca_res_idx,
                 cycles=40, n_lo=1, n_hi=101):
    """Returns (out, per_exec_seconds, info). See docstring in repo history:
    slope between n_lo/n_hi-rep NEFF wall times isolates per-exec device
    time from ~70-110ms axon dispatch overhead."""
    import time

    import jax

    in_maps = _make_in_maps(atom_agg, res_emb, w, b, backbone_idx, ca_res_idx)

    def prep(nc):
        fn, mesh, n_params, in_names, zero_outs = _timed_fn(nc)
        spec = jax.sharding.NamedSharding(mesh, jax.sharding.PartitionSpec("core"))
        per_core = [[np.asarray(m[n]) for n in in_names[:n_params]] for m in in_maps]
        concat = [np.concatenate([per_core[c][i] for c in range(N_CORES)], 0)
                  for i in range(n_params)]
        concat += [np.zeros((N_CORES * z.shape[0], *z.shape[1:]), z.dtype)
                   for z in zero_outs]
        din = [jax.device_put(x, spec) for x in concat]
        outs = fn(*din)
        jax.block_until_ready(outs)  # compile + warm
        return fn, din, outs

    fn_lo, din_lo, outs = prep(_get_nc(n_rep=n_lo))
    fn_hi, din_hi, _ = prep(_get_nc(n_rep=n_hi))

    def timed(fn, din):
        t0 = time.perf_counter()
        jax.block_until_ready(fn(*din))
        return time.perf_counter() - t0

    diffs, ts_lo, ts_hi = [], [], []
    for cyc in range(cycles):
        if cyc % 2 == 0:
            tl = timed(fn_lo, din_lo)
            th = timed(fn_hi, din_hi)
        else:
            th = timed(fn_hi, din_hi)
            tl = timed(fn_lo, din_lo)
        ts_lo.append(tl)
        ts_hi.append(th)
        diffs.append((th - tl) / (n_hi - n_lo))

    diffs = np.array(diffs)
    per_exec = float(np.median(diffs))
    mad = float(np.median(np.abs(diffs - per_exec)))
    q_slopes = [(np.percentile(ts_hi, q) - np.percentile(ts_lo, q))
                / (n_hi - n_lo) for q in (10, 25, 50)]

    ot_len = len(np.asarray(outs[0]).ravel()) // N_CORES
    o = np.asarray(outs[0]).reshape(N_CORES, ot_len)
    results = [{"ot": o[c]} for c in range(N_CORES)]
    out_np = _gather_out(results, b)
    info = {"n": (n_lo, n_hi), "cycles": cycles,
            "paired_median_us": per_exec * 1e6,
            "paired_mad_us": mad * 1e6,
            "quantile_slopes_us": [s * 1e6 for s in q_slopes],
            "lo_ms_q": [float(np.percentile(np.array(ts_lo) * 1e3, q))
                        for q in (5, 25, 50)],
            "hi_ms_q": [float(np.percentile(np.array(ts_hi) * 1e3, q))
                        for q in (5, 25, 50)]}
    est = float(np.median([per_exec] + q_slopes))
    return out_np, est, info


BUILDERS = {
    "v5_wouter": lambda: build_nc(),
}


# revision 5
# speedup vs baseline: 1.5125x; 1.0208x over previous
"""Trainium2 Bass kernel: Atom2Residue (gnn_message_passing).

Math: out[n,c,o] = sum_i fuse[n,c,i] * w[l(c),o,i]  (+ b[o] at c==0)
where fuse[n,c,:] = concat(CA_atom[n,c,:16], res_emb[n,c,:32]), l(c)=floor(sqrt(c)).

Strategy (8 cores, data parallel over residues, no collectives):
  - Host pre-packs a channel-major fp8-E3M4 image ft[432, 18750] per core
    (residues on the free axis; E3M4's 4 mantissa bits keep rel-err at
    1.17e-2 < 2e-2 gate, e4m3 would fail at 2.3e-2), so the device needs
    NO transposes:
      rows   0:128  res channels of coefs 0-3   (g0)
      rows 128:256  res channels of coefs 4-7   (g1)
      rows 256:384  atom channels of coefs 0-7  (g2)
      rows 384:432  res c8 (32) | atom c8 (16)  (g3)
    g0/g1/g2 are interleaved per-partition into ONE [128, 3W] DMA per
    column group; g3 is a second small [48, W] DMA.
  - Device compute is PE-bound (measured, not DMA-bound: the two DMA
    streams alone run 42 us/rep vs 81 us for the naive matmul order), so
    matmuls run WEIGHT-OUTERMOST over chunks of 3 residue-tiles: each of
    the 5 block-diagonal stationary operands (bf16, mixed-dtype matmul
    with the fp8 moving operand) is loaded once per chunk and streams 3
    N=512 matmuls, amortizing the ~160 ns LDWEIGHTS+drain per switch
    (measured: 81 us -> 55 us for the matmul stream).
  - PSUM budget (8 banks): pA0-2 + pB0-2 single-buffered + pC [128,512]
    double-buffered, where chunk tile j's coef-8 output lives at pC
    partitions 32j (one bank for the whole chunk).
  - PSUM -> SBUF bf16 copies split ACT (pA) / DVE (pB, pC); outputs DMA
    out on the ACT ring as a [128, 2W] o01 image + [96, 512*nchunks] o2
    image per group; inputs on the SP ring; triple-buffered.
  - Host un-transposes the output, casts to f32, adds the l=0 bias.
  - HBM traffic/core: 8.1 MB in (fp8) + 10.9 MB out (bf16).
"""

import os
import sys

for _p in ("/opt/trn_rl_repo",):
    if os.path.isdir(_p) and _p not in sys.path:
        sys.path.insert(0, _p)

import numpy as np
from ml_dtypes import bfloat16, float8_e3m4

from concourse import bacc, bass, mybir  # noqa: F401
from concourse.bass_utils import run_bass_kernel_spmd
from concourse.tile import TileContext

F32 = mybir.dt.float32
BF16 = mybir.dt.bfloat16
FP8 = mybir.dt.float8e3

NUM_COEF, ATOM_C, NODE_C = 9, 16, 32
L_OF_COEF = np.floor(np.sqrt(np.arange(NUM_COEF))).astype(np.int64)

N_CORES = 8
R_TOTAL = 150_000
RS = R_TOTAL // N_CORES      # 18750 residues per core
TILE = 512                   # residues per matmul tile (PSUM bank = 512 f32)
RC = RS                      # exact columns per core (no padding)
CH = 3                       # residue-tiles per weight-reuse chunk (PSUM cap)

FT_ROWS = 432                # 128 res(c0-3) + 128 res(c4-7) + 128 atom(c0-7) + 48
WT_COLS = 544                # 128 RA + 128 RB + 128 AA + 128 AB + 32 CC


def _layout(rc=RC):
    """Groups of chunks: first group 1 chunk (small pipeline head), rest 2
    chunks per group. Returns list of groups; each group is a list of chunk
    tile-width lists, e.g. [[512,512,512],[512,512,318]]."""
    nt = -(-rc // TILE)
    tiles = [min(TILE, rc - TILE * t) for t in range(nt)]
    chunks = [tiles[i:i + CH] for i in range(0, nt, CH)]
    groups = [[chunks[0]]]
    i = 1
    while i < len(chunks):
        groups.append(chunks[i:i + 2])
        i += 2
    return groups


def _group_dims(g):
    """(total width, nchunks) of a group."""
    return sum(sum(c) for c in g), len(g)


def build_wt(w):
    """Stationary-weight image [128, 544] bf16. lhsT blocks are [Kin, Mout]."""
    w = np.asarray(w, np.float32)
    wt = np.zeros((128, WT_COLS), np.float32)
    for cl in range(4):
        # RA: res channels of coef cl -> out block cl
        wt[32 * cl:32 * cl + 32, 32 * cl:32 * cl + 32] = \
            w[L_OF_COEF[cl]][:, 16:48].T
        # RB: res channels of coef 4+cl
        wt[32 * cl:32 * cl + 32, 128 + 32 * cl:128 + 32 * cl + 32] = \
            w[L_OF_COEF[4 + cl]][:, 16:48].T
        # AA: atom channels of coef cl (K rows 0:64)
        wt[16 * cl:16 * cl + 16, 256 + 32 * cl:256 + 32 * cl + 32] = \
            w[L_OF_COEF[cl]][:, 0:16].T
        # AB: atom channels of coef 4+cl (K rows 64:128)
        wt[64 + 16 * cl:64 + 16 * cl + 16, 384 + 32 * cl:384 + 32 * cl + 32] = \
            w[L_OF_COEF[4 + cl]][:, 0:16].T
    # CC: coef 8, res (K 0:32) + atom (K 32:48) in one K=48 matmul
    wt[0:32, 512:544] = w[2][:, 16:48].T
    wt[32:48, 512:544] = w[2][:, 0:16].T
    return wt.astype(bfloat16)


def build_nc(rc=RC, n_rep=1, sb_bufs=3, interleave=True):
    """n_rep > 1 statically repeats the whole kernel body inside one NEFF
    (pure timing aid: slope between two n_rep values isolates kernel time
    from the per-dispatch overhead, which is ~70ms >> kernel time here).
    interleave=True alternates the K=64 atom matmuls between PE row-halves
    (T0/T8 are independent array tiles -> LDWEIGHTS overlaps in-flight
    matmuls and the two streams run concurrently)."""
    groups = _layout(rc)
    nc = bacc.Bacc()
    ft_total = FT_ROWS * rc
    ot_total = 256 * rc + 96 * TILE * sum(len(g) for g in groups)
    ft_d = nc.declare_dram_parameter("ft", [ft_total], FP8, isOutput=False)
    wt_d = nc.declare_dram_parameter("wt", [128, WT_COLS], BF16, isOutput=False)
    ot_d = nc.declare_dram_parameter("ot", [ot_total], BF16, isOutput=True)

    with TileContext(nc) as tc:
        with (
            tc.tile_pool(name="const", bufs=1) as cpool,
            tc.tile_pool(name="fin", bufs=sb_bufs) as fin_pool,
            tc.tile_pool(name="osb", bufs=sb_bufs) as osb_pool,
            tc.tile_pool(name="pAB", bufs=1, space="PSUM") as pAB_pool,
            tc.tile_pool(name="pCC", bufs=2, space="PSUM") as pCC_pool,
        ):
            wt_sb = cpool.tile([128, WT_COLS], BF16)
            nc.sync.dma_start(out=wt_sb[:], in_=wt_d[:])

            GW = max(_group_dims(g)[0] for g in groups)
            GNC = max(_group_dims(g)[1] for g in groups)

            for _rep in range(n_rep):
                fb = 0
                ob = 0
                for g in groups:
                    W, nch = _group_dims(g)
                    g012 = fin_pool.tile([128, 3 * GW], FP8, tag="g012")
                    g3 = fin_pool.tile([48, GW], FP8, tag="g3")
                    nc.sync.dma_start(
                        out=g012[:, 0:3 * W],
                        in_=ft_d[fb:fb + 384 * W].rearrange(
                            "(p w) -> p w", w=3 * W))
                    nc.sync.dma_start(
                        out=g3[:, 0:W],
                        in_=ft_d[fb + 384 * W:fb + 432 * W].rearrange(
                            "(p w) -> p w", w=W))
                    fb += 432 * W

                    o01 = osb_pool.tile([128, 2 * GW], BF16, tag="o01")
                    o2 = osb_pool.tile([96, TILE * GNC], BF16, tag="o2")

                    col = 0
                    for k, chunk in enumerate(g):
                        ch = len(chunk)
                        cols = [col + TILE * j for j in range(ch)]
                        pA = [pAB_pool.tile([128, TILE], F32, tag=f"pA{j}",
                                            name=f"pA{j}") for j in range(ch)]
                        pB = [pAB_pool.tile([128, TILE], F32, tag=f"pB{j}",
                                            name=f"pB{j}") for j in range(ch)]
                        pC = pCC_pool.tile([128, TILE], F32, tag="pC")
                        # pass RA: res coefs 0-3 (LDW once per chunk)
                        for j, tw in enumerate(chunk):
                            nc.tensor.matmul(
                                pA[j][:, 0:tw], wt_sb[0:128, 0:128],
                                g012[:, cols[j]:cols[j] + tw],
                                start=True, stop=False,
                                skip_group_check=True, tile_position=(0, 0))
                        # pass RB: res coefs 4-7
                        for j, tw in enumerate(chunk):
                            nc.tensor.matmul(
                                pB[j][:, 0:tw], wt_sb[0:128, 128:256],
                                g012[:, W + cols[j]:W + cols[j] + tw],
                                start=True, stop=False,
                                skip_group_check=True, tile_position=(0, 0))

                        # atom passes: AA (rows 0:64 -> pA) and AB (rows
                        # 64:128 -> pB), interleaved across the two
                        # independent PE row-half tiles
                        def mm_aa(j, tw):
                            nc.tensor.matmul(
                                pA[j][:, 0:tw], wt_sb[0:64, 256:384],
                                g012[0:64, 2 * W + cols[j]:2 * W + cols[j] + tw],
                                start=False, stop=True,
                                skip_group_check=True, tile_position=(0, 0))

                        def mm_ab(j, tw):
                            nc.tensor.matmul(
                                pB[j][:, 0:tw], wt_sb[64:128, 384:512],
                                g012[64:128,
                                     2 * W + cols[j]:2 * W + cols[j] + tw],
                                start=False, stop=True,
                                skip_group_check=True, tile_position=(64, 0))

                        if interleave:
                            for j, tw in enumerate(chunk):
                                mm_aa(j, tw)
                                mm_ab(j, tw)
                        else:
                            for j, tw in enumerate(chunk):
                                mm_aa(j, tw)
                            for j, tw in enumerate(chunk):
                                mm_ab(j, tw)
                        # pass CC: coef 8; chunk tile j -> pC partitions 32j
                        # (independent 32-col groups, concurrent-capable)
                        for j, tw in enumerate(chunk):
                            nc.tensor.matmul(
                                pC[32 * j:32 * j + 32, 0:tw],
                                wt_sb[0:48, 512:544],
                                g3[0:48, cols[j]:cols[j] + tw],
                                start=True, stop=True,
                                skip_group_check=True,
                                tile_position=(0, 32 * j))
                        # PSUM evacuation: pA on ACT, pB + pC on DVE
                        for j, tw in enumerate(chunk):
                            nc.scalar.copy(out=o01[:, cols[j]:cols[j] + tw],
                                           in_=pA[j][:, 0:tw])
                            nc.vector.tensor_copy(
                                o01[:, W + cols[j]:W + cols[j] + tw],
                                pB[j][:, 0:tw])
                        nc.vector.tensor_copy(
                            o2[0:32 * ch, TILE * k:TILE * k + TILE],
                            pC[0:32 * ch, :])
                        col += sum(chunk)

                    # outputs on the second HWDGE ring (ACT); SP carries
                    # only the input stream
                    nc.scalar.dma_start(
                        out=ot_d[ob:ob + 256 * W].rearrange(
                            "(p w) -> p w", w=2 * W),
                        in_=o01[:, 0:2 * W])
                    nc.scalar.dma_start(
                        out=ot_d[ob + 256 * W:ob + 256 * W + 96 * TILE * nch]
                        .rearrange("(p w) -> p w", w=TILE * nch),
                        in_=o2[:, 0:TILE * nch])
                    ob += 256 * W + 96 * TILE * nch
    nc.finalize()
    return nc


_NC_CACHE = {}


def _get_nc(rc=RC, n_rep=1):
    if (rc, n_rep) not in _NC_CACHE:
        _NC_CACHE[(rc, n_rep)] = build_nc(rc, n_rep)
    return _NC_CACHE[(rc, n_rep)]


def _make_in_maps(atom_agg, res_emb, w, b, backbone_idx, ca_res_idx):
    atom_agg = np.asarray(atom_agg)
    res_emb = np.asarray(res_emb)
    backbone_idx = np.asarray(backbone_idx)
    ca_res_idx = np.asarray(ca_res_idx)
    num_res = res_emb.shape[0]
    assert num_res == R_TOTAL, f"kernel compiled for {R_TOTAL} residues"

    wt = build_wt(w)
    E = res_emb.reshape(num_res, 288)
    A = atom_agg.reshape(atom_agg.shape[0], 144)

    ca_atom = backbone_idx.reshape(-1, 4)[:, 1]
    fast = (
        ca_atom.shape[0] == num_res
        and np.array_equal(ca_res_idx, np.arange(num_res, dtype=ca_res_idx.dtype))
        and np.array_equal(ca_atom, 4 * np.arange(num_res, dtype=ca_atom.dtype) + 1)
    )
    if fast:
        CA = A[1::4]                       # strided view [R, 144]
    else:
        CA = np.zeros((num_res, 144), np.float32)
        CA[ca_res_idx] = A[ca_atom]

    groups = _layout(RC)
    in_maps = []
    for c in range(N_CORES):
        r0 = c * RS
        ft = np.empty((FT_ROWS, RC), float8_e3m4)
        Eb = E[r0:r0 + RC].astype(float8_e3m4)     # contiguous cast
        Cb = CA[r0:r0 + RC].astype(float8_e3m4)
        ft[0:256] = Eb.T[0:256]
        ft[256:384] = Cb.T[0:128]
        ft[384:416] = Eb.T[256:288]
        ft[416:432] = Cb.T[128:144]
        # per-group: [128, 3, W] interleave of g0/g1/g2, then [48, W] g3
        blocks = []
        o = 0
        for g in groups:
            W, _ = _group_dims(g)
            m = np.ascontiguousarray(
                ft[0:384, o:o + W].reshape(3, 128, W).transpose(1, 0, 2))
            blocks.append(m.ravel())
            blocks.append(np.ascontiguousarray(ft[384:432, o:o + W]).ravel())
            o += W
        in_maps.append({"ft": np.concatenate(blocks), "wt": wt})
    return in_maps


def _gather_out(results, b):
    groups = _layout(RC)
    out = np.empty((R_TOTAL, 288), np.float32)
    for c in range(N_CORES):
        ot = np.asarray(results[c]["ot"]).ravel()
        ob = 0
        o = 0
        for g in groups:
            W, nch = _group_dims(g)
            blk01 = ot[ob:ob + 256 * W].reshape(128, 2, W)
            blk2 = ot[ob + 256 * W:ob + 256 * W + 96 * TILE * nch].reshape(
                96, TILE * nch)
            r0 = c * RS + o
            out[r0:r0 + W, 0:128] = blk01[:, 0].T
            out[r0:r0 + W, 128:256] = blk01[:, 1].T
            col = 0
            for k, chunk in enumerate(g):
                for j, tw in enumerate(chunk):
                    out[r0 + col:r0 + col + tw, 256:288] = \
                        blk2[32 * j:32 * j + 32,
                             TILE * k:TILE * k + tw].T
                    col += tw
            ob += 256 * W + 96 * TILE * nch
            o += W
    out[:, 0:32] += np.asarray(b, np.float32)[None, :]
    return out.reshape(R_TOTAL, NUM_COEF, NODE_C)


def _run(in_maps, trace=False, **kw):
    nc = _get_nc()
    return run_bass_kernel_spmd(nc, in_maps, core_ids=list(range(N_CORES)),
                                trace=trace, **kw)


def kernel(atom_agg, res_emb, w, b, backbone_idx, ca_res_idx):
    in_maps = _make_in_maps(atom_agg, res_emb, w, b, backbone_idx, ca_res_idx)
    res = _run(in_maps, trace=False)
    return _gather_out(res.results, b)


def _timed_fn(nc, n_loop=1):
    """Build jitted 8-core executor (single NEFF exec per call)."""
    import jax
    from concourse import bass2jax as B

    B.install_neuronx_cc_hook()
    partition_name = nc.partition_id_tensor.name if nc.partition_id_tensor else None
    in_names, out_names, out_avals, zero_outs = [], [], [], []
    import concourse.mybir as mb
    for alloc in nc.m.functions[0].allocations:
        if not isinstance(alloc, mb.MemoryLocationSet):
            continue
        name = alloc.memorylocations[0].name
        if alloc.kind == "ExternalInput":
            if name != partition_name:
                in_names.append(name)
        elif alloc.kind == "ExternalOutput":
            shape = tuple(alloc.tensor_shape)
            dtype = mb.dt.np(alloc.dtype)
            out_avals.append(jax.core.ShapedArray(shape, dtype))
            out_names.append(name)
            zero_outs.append(np.zeros(shape, dtype))
    n_params = len(in_names)
    in_names = in_names + out_names
    if partition_name is not None:
        in_names.append(partition_name)

    assert n_loop == 1, "neuronx_cc_hook supports exactly one bass_exec per jit"

    def _body(*args):
        operands = list(args)
        if partition_name is not None:
            operands.append(B.partition_id_tensor())
        return tuple(B._bass_exec_p.bind(
            *operands,
            out_avals=tuple(out_avals),
            in_names=tuple(in_names),
            out_names=tuple(out_names),
            lowering_input_output_aliases=(),
            sim_require_finite=True,
            sim_require_nnan=True,
            nc=nc,
        ))

    mesh = B.Mesh(np.asarray(jax.devices()[:N_CORES]), ("core",))
    spec = B.PartitionSpec("core")
    fn = jax.jit(
        B.shard_map(_body, mesh=mesh,
                    in_specs=(spec,) * (n_params + len(out_names)),
                    out_specs=(spec,) * len(out_names), check_rep=False),
        keep_unused=True,
    )
    return fn, mesh, n_params, in_names, zero_outs


def kernel_timed(atom_agg, res_emb, w, b, backbone_idx, ca_res_idx,
                 cycles=40, n_lo=1, n_hi=101):
    """Returns (out, per_exec_seconds, info). See docstring in repo history:
    slope between n_lo/n_hi-rep NEFF wall times isolates per-exec device
    time from ~70-110ms axon dispatch overhead."""
    import time

    import jax

    in_maps = _make_in_maps(atom_agg, res_emb, w, b, backbone_idx, ca_res_idx)

    def prep(nc):
        fn, mesh, n_params, in_names, zero_outs = _timed_fn(nc)
        spec = jax.sharding.NamedSharding(mesh, jax.sharding.PartitionSpec("core"))
        per_core = [[np.asarray(m[n]) for n in in_names[:n_params]] for m in in_maps]
        concat = [np.concatenate([per_core[c][i] for c in range(N_CORES)], 0)
                  for i in range(n_params)]
        concat += [np.zeros((N_CORES * z.shape[0], *z.shape[1:]), z.dtype)
                   for z in zero_outs]
        din = [jax.device_put(x, spec) for x in concat]
        outs = fn(*din)
        jax.block_until_ready(outs)  # compile + warm
        return fn, din, outs

    fn_lo, din_lo, outs = prep(_get_nc(n_rep=n_lo))
    fn_hi, din_hi, _ = prep(_get_nc(n_rep=n_hi))

    def timed(fn, din):
        t0 = time.perf_counter()
        jax.block_until_ready(fn(*din))
        return time.perf_counter() - t0

    diffs, ts_lo, ts_hi = [], [], []
    for cyc in range(cycles):
        if cyc % 2 == 0:
            tl = timed(fn_lo, din_lo)
            th = timed(fn_hi, din_hi)
        else:
            th = timed(fn_hi, din_hi)
            tl = timed(fn_lo, din_lo)
        ts_lo.append(tl)
        ts_hi.append(th)
        diffs.append((th - tl) / (n_hi - n_lo))

    diffs = np.array(diffs)
    per_exec = float(np.median(diffs))
    mad = float(np.median(np.abs(diffs - per_exec)))
    q_slopes = [(np.percentile(ts_hi, q) - np.percentile(ts_lo, q))
                / (n_hi - n_lo) for q in (10, 25, 50)]

    ot_len = len(np.asarray(outs[0]).ravel()) // N_CORES
    o = np.asarray(outs[0]).reshape(N_CORES, ot_len)
    results = [{"ot": o[c]} for c in range(N_CORES)]
    out_np = _gather_out(results, b)
    info = {"n": (n_lo, n_hi), "cycles": cycles,
            "paired_median_us": per_exec * 1e6,
            "paired_mad_us": mad * 1e6,
            "quantile_slopes_us": [s * 1e6 for s in q_slopes],
            "lo_ms_q": [float(np.percentile(np.array(ts_lo) * 1e3, q))
                        for q in (5, 25, 50)],
            "hi_ms_q": [float(np.percentile(np.array(ts_hi) * 1e3, q))
                        for q in (5, 25, 50)]}
    est = float(np.median([per_exec] + q_slopes))
    return out_np, est, info


BUILDERS = {
    "v5_wouter": lambda: build_nc(),
}
